# revision 6
# baseline (speedup 1.0000x reference)
"""FFT_Net Trainium2 kernel: per-(b,c) Range DFT (512) + Doppler DFT (256)
as complex GEMMs on the TensorEngine (float32r), fused InstanceNorm.

Data-parallel over the batch dim across 8 NeuronCores. Self-contained:
kernel(**inputs) takes the full inputs, shards internally, returns the
full (16, 32, 512, 256) float32 output.
"""
import sys

sys.path.insert(0, "/opt/trn_rl_repo")

import numpy as np

import concourse.bass as bass  # noqa: F401 (engine types routed via bacc)
import concourse.tile as tile
from concourse import bacc, mybir
from concourse.bass_utils import run_bass_kernel_spmd

B, C, R, D = 16, 16, 512, 256
NCORES = 8
BS = B // NCORES  # batches per core
EPS = 1e-5
N_NORM = R * D  # instance-norm element count per (b, ch)
F32 = mybir.dt.float32
F32R = mybir.dt.float32r
MULT = mybir.AluOpType.mult
ADD = mybir.AluOpType.add
SUB = mybir.AluOpType.subtract
COPY = mybir.ActivationFunctionType.Copy
SQRT = mybir.ActivationFunctionType.Sqrt
SQUARE = mybir.ActivationFunctionType.Square
X_AXIS = mybir.AxisListType.X

# sumsq reduction engine: "ttr" = DVE tensor_tensor_reduce, "act" = ACT Square
# (this walrus build's birverifier rejects InstTensorTensorReduce)
VARIANT = "act"


def build():
    nc = bacc.Bacc(None, target_bir_lowering=False)

    xr_d = nc.dram_tensor("x_real", [BS, C, R, D], F32, kind="ExternalInput")
    xi_d = nc.dram_tensor("x_imag", [BS, C, R, D], F32, kind="ExternalInput")
    wr512_d = nc.dram_tensor("Wr512", [512, 512], F32, kind="ExternalInput")
    wi512_d = nc.dram_tensor("Wi512", [512, 512], F32, kind="ExternalInput")
    nwi512_d = nc.dram_tensor("nWi512", [512, 512], F32, kind="ExternalInput")
    wr256_d = nc.dram_tensor("Wr256", [256, 256], F32, kind="ExternalInput")
    wi256_d = nc.dram_tensor("Wi256", [256, 256], F32, kind="ExternalInput")
    nwi256_d = nc.dram_tensor("nWi256", [256, 256], F32, kind="ExternalInput")
    out_d = nc.dram_tensor("out", [BS, 2 * C, R, D], F32, kind="ExternalOutput")

    with tile.TileContext(nc) as tc:
        with tc.tile_pool(name="wpool", bufs=1) as wpool, \
             tc.tile_pool(name="xpool", bufs=3) as xpool, \
             tc.tile_pool(name="ypool", bufs=2) as ypool, \
             tc.tile_pool(name="zpool", bufs=3) as zpool, \
             tc.tile_pool(name="stpool", bufs=4) as stpool, \
             tc.tile_pool(name="sqpool", bufs=2) as sqpool, \
             tc.tile_pool(name="pspool", bufs=1, space="PSUM") as pspool:

            # --- weights, resident for the whole kernel ---
            w512 = {}
            for nm, dram in (("wr", wr512_d), ("wi", wi512_d), ("nwi", nwi512_d)):
                t = wpool.tile([128, 4, 512], F32R, name=f"w512_{nm}")
                nc.sync.dma_start(
                    out=t,
                    in_=dram[:].bitcast(F32R).rearrange("(k p) n -> p k n", p=128),
                )
                w512[nm] = t
            w256 = {}
            for nm, dram in (("wr", wr256_d), ("wi", wi256_d), ("nwi", nwi256_d)):
                t = wpool.tile([128, 2, 256], F32R, name=f"w256_{nm}")
                nc.sync.dma_start(
                    out=t,
                    in_=dram[:].bitcast(F32R).rearrange("(k p) n -> p k n", p=128),
                )
                w256[nm] = t
            ones_k = wpool.tile([128, 1], F32, name="ones_k")
            nc.vector.memset(ones_k, 1.0)
            ones_m = wpool.tile([1, 128], F32, name="ones_m")
            nc.vector.memset(ones_m, 1.0)
            eps1 = wpool.tile([1, 1], F32, name="eps1")
            nc.vector.memset(eps1, EPS)

            for b in range(BS):
                for c in range(C):
                    # --- load x (b,c): [512,256] -> [128p, 4k, 256] f32r ---
                    xr = xpool.tile([128, 4, 256], F32R, name="xr", tag="xr")
                    nc.sync.dma_start(
                        out=xr,
                        in_=xr_d[b, c].bitcast(F32R).rearrange(
                            "(k p) d -> p k d", p=128),
                    )
                    xi = xpool.tile([128, 4, 256], F32R, name="xi", tag="xi")
                    nc.sync.dma_start(
                        out=xi,
                        in_=xi_d[b, c].bitcast(F32R).rearrange(
                            "(k p) d -> p k d", p=128),
                    )

                    # --- stage 1: yT = (W512 @ x)^T as x^T @ W512 ---
                    # lhsT = x k,m-tile (stationary), rhs = W512 k-slab (moving)
                    yT = {}
                    for m in range(2):
                        for comp in ("r", "i"):
                            ps1 = pspool.tile([128, 512], F32, name="ps1",
                                              tag="ps1", bufs=2)
                            pairs = ([(xr, w512["wr"]), (xi, w512["nwi"])]
                                     if comp == "r" else
                                     [(xr, w512["wi"]), (xi, w512["wr"])])
                            n = 0
                            for src, w in pairs:
                                for k in range(4):
                                    nc.tensor.matmul(
                                        out=ps1,
                                        lhsT=src[:, k, m * 128:(m + 1) * 128],
                                        rhs=w[:, k, :],
                                        start=(n == 0), stop=(n == 7))
                                    n += 1
                            yt = ypool.tile([128, 512], F32R,
                                            name=f"yT_{comp}{m}",
                                            tag=f"yT_{comp}{m}")
                            nc.vector.tensor_copy(out=yt, in_=ps1)
                            yT[(comp, m)] = yt

                    # --- stage 2: z = y @ W256 ; stats fused ---
                    # partials cols: 0-3 sum_r, 4-7 sum_i, 8-11 q_r, 12-15 q_i
                    partials = stpool.tile([128, 16], F32, name="partials",
                                           tag="partials")
                    zt = {}
                    for ci, comp in enumerate(("r", "i")):
                        z = zpool.tile([128, 4, 256], F32, name=f"z_{comp}",
                                       tag=f"z_{comp}")
                        zt[comp] = z
                        pairs = ([("r", w256["wr"]), ("i", w256["nwi"])]
                                 if comp == "r" else
                                 [("r", w256["wi"]), ("i", w256["wr"])])
                        for m2 in range(4):
                            ps2 = pspool.tile([128, 256], F32, name="ps2",
                                              tag="ps2", bufs=4)
                            n = 0
                            for src_comp, w in pairs:
                                for k2 in range(2):
                                    nc.tensor.matmul(
                                        out=ps2,
                                        lhsT=yT[(src_comp, k2)][
                                            :, m2 * 128:(m2 + 1) * 128],
                                        rhs=w[:, k2, :],
                                        start=(n == 0), stop=(n == 3))
                                    n += 1
                            # PSUM -> SBUF copy + row-sum on ACT
                            col = ci * 4 + m2
                            nc.scalar.activation(
                                out=z[:, m2, :], in_=ps2, func=COPY,
                                accum_out=partials[:, col:col + 1])
                            # row-sum of squares (reads PSUM directly)
                            sq = sqpool.tile([128, 256], F32, name="sq",
                                             tag="sq")
                            if VARIANT == "ttr":
                                nc.vector.tensor_tensor_reduce(
                                    out=sq, in0=ps2, in1=ps2, scale=1.0,
                                    scalar=0.0, op0=MULT, op1=ADD,
                                    accum_out=partials[:, 8 + col:9 + col])
                            else:
                                nc.scalar.activation(
                                    out=sq, in_=ps2, func=SQUARE,
                                    accum_out=partials[:, 8 + col:9 + col])

                    # --- cross-partition reduce via ones-matmul ---
                    pstat = pspool.tile([1, 16], F32, name="pstat",
                                        tag="pstat", bufs=1)
                    nc.tensor.matmul(out=pstat, lhsT=ones_k, rhs=partials,
                                     start=True, stop=True)
                    # stats4 = (S_r, S_i, Q_r, Q_i)
                    stats4 = stpool.tile([1, 4], F32, name="stats4",
                                         tag="stats4")
                    nc.vector.tensor_reduce(
                        out=stats4,
                        in_=pstat.rearrange("p (g m) -> p g m", m=4),
                        axis=X_AXIS, op=ADD)
                    # t4 = (mean_r, mean_i, E2_r, E2_i)
                    t4 = stpool.tile([1, 4], F32, name="t4", tag="t4")
                    nc.vector.tensor_scalar_mul(out=t4, in0=stats4,
                                                scalar1=1.0 / N_NORM)
                    msq = stpool.tile([1, 2], F32, name="msq", tag="msq")
                    nc.vector.tensor_mul(out=msq, in0=t4[:, 0:2],
                                         in1=t4[:, 0:2])
                    var2 = stpool.tile([1, 2], F32, name="var2", tag="var2")
                    nc.vector.tensor_sub(out=var2, in0=t4[:, 2:4], in1=msq)
                    std2 = stpool.tile([1, 2], F32, name="std2", tag="std2")
                    nc.scalar.activation(out=std2, in_=var2, func=SQRT,
                                         bias=eps1, scale=1.0)
                    # ab4 = (istd_r, istd_i, mb_r, mb_i); mb = mean * istd
                    ab4 = stpool.tile([1, 4], F32, name="ab4", tag="ab4")
                    nc.vector.reciprocal(out=ab4[:, 0:2], in_=std2)
                    nc.vector.tensor_mul(out=ab4[:, 2:4], in0=t4[:, 0:2],
                                         in1=ab4[:, 0:2])
                    # broadcast to all partitions via K=1 ones-matmul
                    pab = pspool.tile([128, 4], F32, name="pab", tag="pab",
                                      bufs=1)
                    nc.tensor.matmul(out=pab, lhsT=ones_m, rhs=ab4,
                                     start=True, stop=True)
                    absb = stpool.tile([128, 4], F32, name="absb", tag="absb")
                    nc.vector.tensor_copy(out=absb, in_=pab)

                    # --- normalize in place and store ---
                    for ci, comp in enumerate(("r", "i")):
                        z = zt[comp]
                        nc.vector.tensor_scalar(
                            out=z, in0=z,
                            scalar1=absb[:, ci:ci + 1],
                            scalar2=absb[:, 2 + ci:3 + ci],
                            op0=MULT, op1=SUB)
                        ch = c if comp == "r" else C + c
                        nc.sync.dma_start(
                            out=out_d[b, ch].rearrange("(k p) d -> p k d",
                                                       p=128),
                            in_=z)

    nc.finalize()
    return nc


_NC_CACHE = None


def _get_nc():
    global _NC_CACHE
    if _NC_CACHE is None:
        _NC_CACHE = build()
    return _NC_CACHE


def make_in_maps(inputs):
    xr = np.ascontiguousarray(np.asarray(inputs["x_real"], dtype=np.float32))
    xi = np.ascontiguousarray(np.asarray(inputs["x_imag"], dtype=np.float32))
    wr512 = np.ascontiguousarray(np.asarray(inputs["Wr512"], dtype=np.float32))
    wi512 = np.ascontiguousarray(np.asarray(inputs["Wi512"], dtype=np.float32))
    wr256 = np.ascontiguousarray(np.asarray(inputs["Wr256"], dtype=np.float32))
    wi256 = np.ascontiguousarray(np.asarray(inputs["Wi256"], dtype=np.float32))
    nwi512 = np.ascontiguousarray(-wi512)
    nwi256 = np.ascontiguousarray(-wi256)
    in_maps = []
    for i in range(NCORES):
        in_maps.append({
            "x_real": np.ascontiguousarray(xr[i * BS:(i + 1) * BS]),
            "x_imag": np.ascontiguousarray(xi[i * BS:(i + 1) * BS]),
            "Wr512": wr512, "Wi512": wi512, "nWi512": nwi512,
            "Wr256": wr256, "Wi256": wi256, "nWi256": nwi256,
        })
    return in_maps


def run(inputs, trace=False):
    nc = _get_nc()
    res = run_bass_kernel_spmd(nc, make_in_maps(inputs),
                               list(range(NCORES)), trace=trace)
    out = np.concatenate([res.results[i]["out"] for i in range(NCORES)],
                         axis=0)
    return out, res


def kernel(**inputs):
    out, _ = run(inputs, trace=False)
    return out


if __name__ == "__main__":
    rng = np.random.default_rng(0)
    ins = {
        "x_real": rng.standard_normal((B, C, R, D)).astype(np.float32),
        "x_imag": rng.standard_normal((B, C, R, D)).astype(np.float32),
    }
    n = np.arange(512)
    W = np.exp(-2j * np.pi * np.outer(n, n) / 512).astype(np.complex64)
    ins["Wr512"], ins["Wi512"] = W.real.copy(), W.imag.copy()
    n = np.arange(256)
    W = np.exp(-2j * np.pi * np.outer(n, n) / 256).astype(np.complex64)
    ins["Wr256"], ins["Wi256"] = W.real.copy(), W.imag.copy()
    out = kernel(**ins)
    print("out", out.shape, out.dtype, float(np.abs(out).mean()))


# revision 12
# speedup vs baseline: 1.0033x; 1.0033x over previous
"""FFT_Net Trainium2 kernel: per-(b,c) Range DFT (512) + Doppler DFT (256)
as complex GEMMs on the TensorEngine (float32r), fused InstanceNorm.

Data-parallel over the batch dim across 8 NeuronCores. Self-contained:
kernel(**inputs) takes the full inputs, shards internally, returns the
full (16, 32, 512, 256) float32 output.
"""
import sys

sys.path.insert(0, "/opt/trn_rl_repo")

import numpy as np

import concourse.bass as bass  # noqa: F401 (engine types routed via bacc)
import concourse.tile as tile
from concourse import bacc, mybir
from concourse.bass_utils import run_bass_kernel_spmd

B, C, R, D = 16, 16, 512, 256
NCORES = 8
BS = B // NCORES  # batches per core
EPS = 1e-5
N_NORM = R * D  # instance-norm element count per (b, ch)
F32 = mybir.dt.float32
F32R = mybir.dt.float32r
MULT = mybir.AluOpType.mult
ADD = mybir.AluOpType.add
SUB = mybir.AluOpType.subtract
COPY = mybir.ActivationFunctionType.Copy
SQRT = mybir.ActivationFunctionType.Sqrt
SQUARE = mybir.ActivationFunctionType.Square
X_AXIS = mybir.AxisListType.X

# sumsq reduction engine: "ttr" = DVE tensor_tensor_reduce, "act" = ACT Square
# (this walrus build's birverifier rejects InstTensorTensorReduce)
VARIANT = "act"


def build():
    nc = bacc.Bacc(None, target_bir_lowering=False)

    xr_d = nc.dram_tensor("x_real", [BS, C, R, D], F32, kind="ExternalInput")
    xi_d = nc.dram_tensor("x_imag", [BS, C, R, D], F32, kind="ExternalInput")
    wr512_d = nc.dram_tensor("Wr512", [512, 512], F32, kind="ExternalInput")
    wi512_d = nc.dram_tensor("Wi512", [512, 512], F32, kind="ExternalInput")
    nwi512_d = nc.dram_tensor("nWi512", [512, 512], F32, kind="ExternalInput")
    # stage-2 weights come pre-concatenated from the host:
    # catA = [Wr256 | Wi256], catB = [-Wi256 | Wr256]  (both [256, 512])
    w256a_d = nc.dram_tensor("W256catA", [256, 512], F32, kind="ExternalInput")
    w256b_d = nc.dram_tensor("W256catB", [256, 512], F32, kind="ExternalInput")
    out_d = nc.dram_tensor("out", [BS, 2 * C, R, D], F32, kind="ExternalOutput")

    with tile.TileContext(nc) as tc:
        with tc.tile_pool(name="wpool", bufs=1) as wpool, \
             tc.tile_pool(name="xpool", bufs=3) as xpool, \
             tc.tile_pool(name="ypool", bufs=2) as ypool, \
             tc.tile_pool(name="zpool", bufs=3) as zpool, \
             tc.tile_pool(name="stpool", bufs=4) as stpool, \
             tc.tile_pool(name="sqpool", bufs=2) as sqpool, \
             tc.tile_pool(name="pspool", bufs=1, space="PSUM") as pspool:

            # --- weights, resident for the whole kernel ---
            w512 = {}
            for nm, dram in (("wr", wr512_d), ("wi", wi512_d), ("nwi", nwi512_d)):
                t = wpool.tile([128, 4, 512], F32R, name=f"w512_{nm}")
                nc.sync.dma_start(
                    out=t,
                    in_=dram[:].bitcast(F32R).rearrange("(k p) n -> p k n", p=128),
                )
                w512[nm] = t
            w256 = {}
            for nm, dram in (("a", w256a_d), ("b", w256b_d)):
                t = wpool.tile([128, 2, 512], F32R, name=f"w256_{nm}")
                nc.sync.dma_start(
                    out=t,
                    in_=dram[:].bitcast(F32R).rearrange("(k p) n -> p k n", p=128),
                )
                w256[nm] = t
            ones_k = wpool.tile([128, 1], F32, name="ones_k")
            nc.vector.memset(ones_k, 1.0)
            ones_m = wpool.tile([1, 128], F32, name="ones_m")
            nc.vector.memset(ones_m, 1.0)
            eps1 = wpool.tile([1, 1], F32, name="eps1")
            nc.vector.memset(eps1, EPS)

            for b in range(BS):
                for c in range(C):
                    # --- load x (b,c): [512,256] -> [128p, 4k, 256] f32r ---
                    xr = xpool.tile([128, 4, 256], F32R, name="xr", tag="xr")
                    nc.sync.dma_start(
                        out=xr,
                        in_=xr_d[b, c].bitcast(F32R).rearrange(
                            "(k p) d -> p k d", p=128),
                    )
                    xi = xpool.tile([128, 4, 256], F32R, name="xi", tag="xi")
                    nc.sync.dma_start(
                        out=xi,
                        in_=xi_d[b, c].bitcast(F32R).rearrange(
                            "(k p) d -> p k d", p=128),
                    )

                    # --- stage 1: yT = (W512 @ x)^T as x^T @ W512 ---
                    # lhsT = x k,m-tile (stationary), rhs = W512 k-slab (moving)
                    yT = {}
                    for m in range(2):
                        for comp in ("r", "i"):
                            ps1 = pspool.tile([128, 512], F32, name="ps1",
                                              tag="ps1", bufs=2)
                            pairs = ([(xr, w512["wr"]), (xi, w512["nwi"])]
                                     if comp == "r" else
                                     [(xr, w512["wi"]), (xi, w512["wr"])])
                            n = 0
                            for src, w in pairs:
                                for k in range(4):
                                    nc.tensor.matmul(
                                        out=ps1,
                                        lhsT=src[:, k, m * 128:(m + 1) * 128],
                                        rhs=w[:, k, :],
                                        start=(n == 0), stop=(n == 7))
                                    n += 1
                            yt = ypool.tile([128, 512], F32R,
                                            name=f"yT_{comp}{m}",
                                            tag=f"yT_{comp}{m}")
                            nc.scalar.copy(out=yt, in_=ps1)
                            yT[(comp, m)] = yt

                    # --- stage 2: [zr | zi] = y @ [W256catA ; W256catB] ---
                    # per m2 one psum bank [128, 512]: cols 0-255 zr, 256- zi
                    # partials cols: 0-3 sum_r, 4-7 sum_i, 8-11 q_r, 12-15 q_i
                    partials = stpool.tile([128, 16], F32, name="partials",
                                           tag="partials")
                    z_r = zpool.tile([128, 4, 256], F32, name="z_r", tag="z_r")
                    z_i = zpool.tile([128, 4, 256], F32, name="z_i", tag="z_i")
                    zt = {"r": z_r, "i": z_i}
                    for m2 in range(4):
                        ps2 = pspool.tile([128, 512], F32, name="ps2",
                                          tag="ps2", bufs=4)
                        n = 0
                        for src_comp, w in (("r", w256["a"]), ("i", w256["b"])):
                            for k2 in range(2):
                                nc.tensor.matmul(
                                    out=ps2,
                                    lhsT=yT[(src_comp, k2)][
                                        :, m2 * 128:(m2 + 1) * 128],
                                    rhs=w[:, k2, :],
                                    start=(n == 0), stop=(n == 3))
                                n += 1
                        for ci, comp in enumerate(("r", "i")):
                            half = ps2[:, ci * 256:(ci + 1) * 256]
                            col = ci * 4 + m2
                            # PSUM -> SBUF copy + row-sum on DVE
                            nc.vector.tensor_scalar(
                                out=zt[comp][:, m2, :], in0=half,
                                scalar1=1.0, scalar2=0.0, op0=MULT, op1=ADD,
                                accum_out=partials[:, col:col + 1])
                            # row-sum of squares on ACT
                            sq = sqpool.tile([128, 256], F32, name="sq",
                                             tag="sq")
                            nc.scalar.activation(
                                out=sq, in_=half, func=SQUARE,
                                accum_out=partials[:, 8 + col:9 + col])

                    # --- cross-partition reduce via ones-matmul ---
                    pstat = pspool.tile([1, 16], F32, name="pstat",
                                        tag="pstat", bufs=1)
                    nc.tensor.matmul(out=pstat, lhsT=ones_k, rhs=partials,
                                     start=True, stop=True)
                    # stats4 = (S_r, S_i, Q_r, Q_i)
                    stats4 = stpool.tile([1, 4], F32, name="stats4",
                                         tag="stats4")
                    nc.vector.tensor_reduce(
                        out=stats4,
                        in_=pstat.rearrange("p (g m) -> p g m", m=4),
                        axis=X_AXIS, op=ADD)
                    # t4 = (mean_r, mean_i, E2_r, E2_i)
                    t4 = stpool.tile([1, 4], F32, name="t4", tag="t4")
                    nc.vector.tensor_scalar_mul(out=t4, in0=stats4,
                                                scalar1=1.0 / N_NORM)
                    msq = stpool.tile([1, 2], F32, name="msq", tag="msq")
                    nc.vector.tensor_mul(out=msq, in0=t4[:, 0:2],
                                         in1=t4[:, 0:2])
                    var2 = stpool.tile([1, 2], F32, name="var2", tag="var2")
                    nc.vector.tensor_sub(out=var2, in0=t4[:, 2:4], in1=msq)
                    std2 = stpool.tile([1, 2], F32, name="std2", tag="std2")
                    nc.scalar.activation(out=std2, in_=var2, func=SQRT,
                                         bias=eps1, scale=1.0)
                    # ab4 = (istd_r, istd_i, mb_r, mb_i); mb = mean * istd
                    ab4 = stpool.tile([1, 4], F32, name="ab4", tag="ab4")
                    nc.vector.reciprocal(out=ab4[:, 0:2], in_=std2)
                    nc.vector.tensor_mul(out=ab4[:, 2:4], in0=t4[:, 0:2],
                                         in1=ab4[:, 0:2])
                    # broadcast to all partitions via K=1 ones-matmul
                    pab = pspool.tile([128, 4], F32, name="pab", tag="pab",
                                      bufs=1)
                    nc.tensor.matmul(out=pab, lhsT=ones_m, rhs=ab4,
                                     start=True, stop=True)
                    absb = stpool.tile([128, 4], F32, name="absb", tag="absb")
                    nc.vector.tensor_copy(out=absb, in_=pab)

                    # --- normalize in place and store ---
                    for ci, comp in enumerate(("r", "i")):
                        z = zt[comp]
                        nc.vector.tensor_scalar(
                            out=z, in0=z,
                            scalar1=absb[:, ci:ci + 1],
                            scalar2=absb[:, 2 + ci:3 + ci],
                            op0=MULT, op1=SUB)
                        ch = c if comp == "r" else C + c
                        nc.sync.dma_start(
                            out=out_d[b, ch].rearrange("(k p) d -> p k d",
                                                       p=128),
                            in_=z)

    nc.finalize()
    return nc


_NC_CACHE = None


def _get_nc():
    global _NC_CACHE
    if _NC_CACHE is None:
        _NC_CACHE = build()
    return _NC_CACHE


def make_in_maps(inputs):
    xr = np.ascontiguousarray(np.asarray(inputs["x_real"], dtype=np.float32))
    xi = np.ascontiguousarray(np.asarray(inputs["x_imag"], dtype=np.float32))
    wr512 = np.ascontiguousarray(np.asarray(inputs["Wr512"], dtype=np.float32))
    wi512 = np.ascontiguousarray(np.asarray(inputs["Wi512"], dtype=np.float32))
    wr256 = np.ascontiguousarray(np.asarray(inputs["Wr256"], dtype=np.float32))
    wi256 = np.ascontiguousarray(np.asarray(inputs["Wi256"], dtype=np.float32))
    nwi512 = np.ascontiguousarray(-wi512)
    w256a = np.ascontiguousarray(np.concatenate([wr256, wi256], axis=1))
    w256b = np.ascontiguousarray(np.concatenate([-wi256, wr256], axis=1))
    in_maps = []
    for i in range(NCORES):
        in_maps.append({
            "x_real": np.ascontiguousarray(xr[i * BS:(i + 1) * BS]),
            "x_imag": np.ascontiguousarray(xi[i * BS:(i + 1) * BS]),
            "Wr512": wr512, "Wi512": wi512, "nWi512": nwi512,
            "W256catA": w256a, "W256catB": w256b,
        })
    return in_maps


def run(inputs, trace=False):
    nc = _get_nc()
    res = run_bass_kernel_spmd(nc, make_in_maps(inputs),
                               list(range(NCORES)), trace=trace)
    out = np.concatenate([res.results[i]["out"] for i in range(NCORES)],
                         axis=0)
    return out, res


def kernel(**inputs):
    out, _ = run(inputs, trace=False)
    return out


if __name__ == "__main__":
    rng = np.random.default_rng(0)
    ins = {
        "x_real": rng.standard_normal((B, C, R, D)).astype(np.float32),
        "x_imag": rng.standard_normal((B, C, R, D)).astype(np.float32),
    }
    n = np.arange(512)
    W = np.exp(-2j * np.pi * np.outer(n, n) / 512).astype(np.complex64)
    ins["Wr512"], ins["Wi512"] = W.real.copy(), W.imag.copy()
    n = np.arange(256)
    W = np.exp(-2j * np.pi * np.outer(n, n) / 256).astype(np.complex64)
    ins["Wr256"], ins["Wi256"] = W.real.copy(), W.imag.copy()
    out = kernel(**ins)
    print("out", out.shape, out.dtype, float(np.abs(out).mean()))
